# revision 17
# baseline (speedup 1.0000x reference)
"""Bilinear interpolation (spatial transformer sampling) on 8 TRN2 NeuronCores.

Per core (4 batches, pure data parallel):
  1. Stage a gather table TBL per batch in DRAM: two planes of 256-B entries
     (4 f32 pixels each). plane0 = the image; plane1 = the image shifted by
     2 pixels. Both are single linear DMA copies.
  2. Compute affine coords/weights per output point on DVE.
  3. dma_gather (SWDGE) one 256-B entry per (point, stencil row):
     idx = sel*16384 + y0*64 + (x0>>2) with sel = bit1(x0); the wanted pixel
     pair then sits at entry slots {d, d+1}, d = x0&1 in {0,1}.
  4. 3-slot weighted combine per row + y-blend on DVE, masked for OOB.

Point layout: t = p*392 + c (p = partition, c = global column). A gather
call covers columns [k*CC, (k+1)*CC); gathered tile position (p, c_loc)
holds gather-id g = c_loc*128 + p. dma_gather reads indices from a
16-partition-wrapped buffer (idx of g at [g%16, g//16], replicated on all
8 16-partition groups).
"""

import numpy as np

from concourse import bacc, bass, mybir
from concourse.bass_utils import run_bass_kernel_spmd
from concourse.tile import TileContext

B, H, W, C = 32, 256, 256, 16
OUT_H = OUT_W = 224
P = OUT_H * OUT_W            # 50176
NCORES = 8
BLOC = B // NCORES           # 4 batches per core
NPART = 128
NCOL = P // NPART            # 392
NCHUNK = 7
CCOL = NCOL // NCHUNK        # 56 columns per chunk
HWPIX = H * W                # 65536
NENT = 2 * 16384             # table entries (2 planes x 256 rows x 64)

f32 = mybir.dt.float32
i16 = mybir.dt.int16
Alu = mybir.AluOpType


def build_program() -> bass.Bass:
    nc = bacc.Bacc("TRN2")
    tbls = [
        nc.declare_dram_parameter(f"tbl{i}", [NENT, 64], f32, isOutput=False)
        for i in range(BLOC)
    ]
    theta = nc.declare_dram_parameter("theta", [NPART, BLOC * 6], f32, isOutput=False)
    ug = nc.declare_dram_parameter("ug", [NPART, NCOL], f32, isOutput=False)
    vg = nc.declare_dram_parameter("vg", [NPART, NCOL], f32, isOutput=False)
    out = nc.declare_dram_parameter("out", [BLOC * P, C], f32, isOutput=True)
    out_r = out.rearrange("(b p n) c -> b p n c", b=BLOC, p=NPART, n=NCOL)

    with TileContext(nc) as tc:
        with (
            tc.tile_pool(name="const", bufs=1) as cpool,
            tc.tile_pool(name="scratch", bufs=1) as spool,
            tc.tile_pool(name="persist", bufs=2) as ppool,
            tc.tile_pool(name="gather", bufs=2) as gpool,
            tc.tile_pool(name="result", bufs=3) as rpool,
        ):
            ug_s = cpool.tile([NPART, NCOL], f32, tag="ug")
            vg_s = cpool.tile([NPART, NCOL], f32, tag="vg")
            nc.sync.dma_start(out=ug_s[:], in_=ug[:])
            nc.sync.dma_start(out=vg_s[:], in_=vg[:])

            for b in range(BLOC):
                tblv = tbls[b]

                # ---- per-batch affine coefficients (broadcast via DMA) ----
                th = spool.tile([NPART, 6], f32, tag="th", name="th")
                nc.sync.dma_start(out=th[:], in_=theta[:, 6 * b : 6 * b + 6])
                # theta row-major [t00 t01 t02 t10 t11 t12]
                # x_pix = 128*t00*u + 128*t01*v + (128*t02 + 128)
                coef = spool.tile([NPART, 6], f32, tag="coef", name="coef")
                nc.vector.tensor_scalar(
                    out=coef[:], in0=th[:], scalar1=128.0, scalar2=None, op0=Alu.mult
                )
                nc.vector.tensor_scalar(
                    out=coef[:, 2:3], in0=th[:, 2:3], scalar1=128.0, scalar2=128.0,
                    op0=Alu.mult, op1=Alu.add,
                )
                nc.vector.tensor_scalar(
                    out=coef[:, 5:6], in0=th[:, 5:6], scalar1=128.0, scalar2=128.0,
                    op0=Alu.mult, op1=Alu.add,
                )
                ax, bx, cx = coef[:, 0:1], coef[:, 1:2], coef[:, 2:3]
                ay, by, cy = coef[:, 3:4], coef[:, 4:5], coef[:, 5:6]

                def tile392(tag):
                    return spool.tile([NPART, NCOL], f32, tag=tag, name=tag)

                x = tile392("x")
                y = tile392("y")
                t2 = tile392("t2")
                nc.vector.tensor_scalar(out=x[:], in0=ug_s[:], scalar1=ax, scalar2=cx,
                                        op0=Alu.mult, op1=Alu.add)
                nc.vector.tensor_scalar(out=t2[:], in0=vg_s[:], scalar1=bx,
                                        scalar2=None, op0=Alu.mult)
                nc.vector.tensor_add(out=x[:], in0=x[:], in1=t2[:])
                t3 = tile392("t3")
                nc.vector.tensor_scalar(out=y[:], in0=ug_s[:], scalar1=ay, scalar2=cy,
                                        op0=Alu.mult, op1=Alu.add)
                nc.vector.tensor_scalar(out=t3[:], in0=vg_s[:], scalar1=by,
                                        scalar2=None, op0=Alu.mult)
                nc.vector.tensor_add(out=y[:], in0=y[:], in1=t3[:])

                # clamp to [0,254]; integer/frac split (mod works: args >= 0)
                xc = tile392("xc")
                yc = tile392("yc")
                nc.vector.tensor_scalar(out=xc[:], in0=x[:], scalar1=0.0, scalar2=254.0,
                                        op0=Alu.max, op1=Alu.min)
                nc.vector.tensor_scalar(out=yc[:], in0=y[:], scalar1=0.0, scalar2=254.0,
                                        op0=Alu.max, op1=Alu.min)
                # floor via int roundtrip + compare correction (no mod in ISA)
                xi = spool.tile([NPART, NCOL], mybir.dt.int32, tag="xi", name="xi")
                xf = tile392("xf")
                gtx = tile392("gtx")
                x0f = tile392("x0f")
                nc.vector.tensor_copy(out=xi[:], in_=xc[:])
                nc.vector.tensor_copy(out=xf[:], in_=xi[:])
                nc.vector.tensor_tensor(out=gtx[:], in0=xf[:], in1=xc[:],
                                        op=Alu.is_gt)
                nc.vector.tensor_sub(out=x0f[:], in0=xf[:], in1=gtx[:])
                yi = spool.tile([NPART, NCOL], mybir.dt.int32, tag="yi", name="yi")
                yf = tile392("yf")
                gty = tile392("gty")
                y0f = tile392("y0f")
                nc.vector.tensor_copy(out=yi[:], in_=yc[:])
                nc.vector.tensor_copy(out=yf[:], in_=yi[:])
                nc.vector.tensor_tensor(out=gty[:], in0=yf[:], in1=yc[:],
                                        op=Alu.is_gt)
                nc.vector.tensor_sub(out=y0f[:], in0=yf[:], in1=gty[:])

                wx1 = tile392("wx1")
                wy1 = tile392("wy1")
                nc.vector.tensor_sub(out=wx1[:], in0=x[:], in1=x0f[:])
                nc.vector.tensor_sub(out=wy1[:], in0=y[:], in1=y0f[:])
                wx0 = tile392("wx0")
                wy0 = tile392("wy0")
                nc.vector.tensor_scalar(out=wx0[:], in0=wx1[:], scalar1=-1.0,
                                        scalar2=1.0, op0=Alu.mult, op1=Alu.add)
                nc.vector.tensor_scalar(out=wy0[:], in0=wy1[:], scalar1=-1.0,
                                        scalar2=1.0, op0=Alu.mult, op1=Alu.add)

                # OOB zero mask: nonzero iff -1 < x < 255 and -1 < y < 255
                m1 = tile392("m1")
                m2 = tile392("m2")
                mask = tile392("mask")
                nc.vector.tensor_scalar(out=m1[:], in0=x[:], scalar1=-1.0,
                                        scalar2=None, op0=Alu.is_gt)
                nc.vector.tensor_scalar(out=m2[:], in0=x[:], scalar1=255.0,
                                        scalar2=None, op0=Alu.is_lt)
                nc.vector.tensor_mul(out=mask[:], in0=m1[:], in1=m2[:])
                nc.vector.tensor_scalar(out=m1[:], in0=y[:], scalar1=-1.0,
                                        scalar2=None, op0=Alu.is_gt)
                nc.vector.tensor_mul(out=mask[:], in0=mask[:], in1=m1[:])
                nc.vector.tensor_scalar(out=m2[:], in0=y[:], scalar1=255.0,
                                        scalar2=None, op0=Alu.is_lt)
                nc.vector.tensor_mul(out=mask[:], in0=mask[:], in1=m2[:])

                wy0m = tile392("wy0m")
                wy1m = tile392("wy1m")
                nc.vector.tensor_mul(out=wy0m[:], in0=wy0[:], in1=mask[:])
                nc.vector.tensor_mul(out=wy1m[:], in0=wy1[:], in1=mask[:])

                # entry slot weights: d = x0 mod 2 selects slots {0,1} or {1,2}
                # m4 = x0 mod 4 via floor(x0/4); jx = x0>>2 falls out free
                q = tile392("q")
                nc.vector.tensor_scalar(out=q[:], in0=x0f[:], scalar1=0.25,
                                        scalar2=None, op0=Alu.mult)
                nc.vector.tensor_copy(out=xi[:], in_=q[:])
                qf = tile392("qf")
                nc.vector.tensor_copy(out=qf[:], in_=xi[:])
                gtq = tile392("gtq")
                nc.vector.tensor_tensor(out=gtq[:], in0=qf[:], in1=q[:],
                                        op=Alu.is_gt)
                jx = tile392("jx")
                nc.vector.tensor_sub(out=jx[:], in0=qf[:], in1=gtq[:])
                m4 = tile392("m4")
                nc.vector.tensor_scalar(out=m4[:], in0=jx[:], scalar1=-4.0,
                                        scalar2=None, op0=Alu.mult)
                nc.vector.tensor_add(out=m4[:], in0=m4[:], in1=x0f[:])
                sel = tile392("sel")
                nc.vector.tensor_scalar(out=sel[:], in0=m4[:], scalar1=2.0,
                                        scalar2=None, op0=Alu.is_ge)
                d = tile392("d")
                nc.vector.tensor_scalar(out=d[:], in0=sel[:], scalar1=-2.0,
                                        scalar2=None, op0=Alu.mult)
                nc.vector.tensor_add(out=d[:], in0=d[:], in1=m4[:])
                md0 = tile392("md0")
                nc.vector.tensor_scalar(out=md0[:], in0=d[:], scalar1=-1.0,
                                        scalar2=1.0, op0=Alu.mult, op1=Alu.add)
                wq0 = tile392("wq0")
                wq2 = tile392("wq2")
                wq1 = tile392("wq1")
                nc.vector.tensor_mul(out=wq0[:], in0=wx0[:], in1=md0[:])
                nc.vector.tensor_mul(out=wq2[:], in0=wx1[:], in1=d[:])
                nc.vector.tensor_add(out=wq1[:], in0=wq0[:], in1=wq2[:])
                nc.vector.tensor_scalar(out=wq1[:], in0=wq1[:], scalar1=-1.0,
                                        scalar2=1.0, op0=Alu.mult, op1=Alu.add)

                # final 6 weights (persist through chunk loop)
                Wt = []
                for r, wyr in ((0, wy0m), (1, wy1m)):
                    for m, wqm in ((0, wq0), (1, wq1), (2, wq2)):
                        w = ppool.tile([NPART, NCOL], f32, tag=f"W{r}{m}",
                                       name=f"W{r}{m}")
                        nc.vector.tensor_mul(out=w[:], in0=wyr[:], in1=wqm[:])
                        Wt.append(w)

                # gather indices: iq1 = sel*16384 + y0*64 + jx
                iq1 = tile392("iq1")
                nc.vector.tensor_scalar(out=iq1[:], in0=y0f[:], scalar1=64.0,
                                        scalar2=None, op0=Alu.mult)
                nc.vector.tensor_add(out=iq1[:], in0=iq1[:], in1=jx[:])
                nc.vector.tensor_scalar(out=t2[:], in0=sel[:], scalar1=16384.0,
                                        scalar2=None, op0=Alu.mult)
                nc.vector.tensor_add(out=iq1[:], in0=iq1[:], in1=t2[:])
                iq2 = tile392("iq2")
                nc.vector.tensor_scalar(out=iq2[:], in0=iq1[:], scalar1=64.0,
                                        scalar2=None, op0=Alu.add)

                # int16 + fold into 16-partition wrapped layout, replicated x8.
                # wrapped[q, c*8 + r] = iq[16*r + q, c]
                iqs1 = spool.tile([NPART, NCOL], i16, tag="iqs1", name="iqs1")
                iqs2 = spool.tile([NPART, NCOL], i16, tag="iqs2", name="iqs2")
                nc.vector.tensor_copy(out=iqs1[:], in_=iq1[:])
                nc.vector.tensor_copy(out=iqs2[:], in_=iq2[:])
                # partition-shift blocks of 16 rows down to partitions 0..15
                tmp1 = spool.tile([16, 8, NCOL], i16, tag="tmp1", name="tmp1")
                tmp2 = spool.tile([16, 8, NCOL], i16, tag="tmp2", name="tmp2")
                for r in range(8):
                    nc.sync.dma_start(out=tmp1[0:16, r, :],
                                      in_=iqs1[16 * r : 16 * r + 16, :])
                    nc.sync.dma_start(out=tmp2[0:16, r, :],
                                      in_=iqs2[16 * r : 16 * r + 16, :])
                # interleave into wrapped layout (within partitions 0..15);
                # contiguous write + strided read (strided writes lower badly)
                w1 = ppool.tile([NPART, NCOL, 8], i16, tag="w1", name="w1")
                w2 = ppool.tile([NPART, NCOL, 8], i16, tag="w2", name="w2")
                nc.vector.tensor_copy(
                    out=w1[0:16, :, :],
                    in_=tmp1[0:16, :, :].rearrange("p r n -> p n r"))
                nc.vector.tensor_copy(
                    out=w2[0:16, :, :],
                    in_=tmp2[0:16, :, :].rearrange("p r n -> p n r"))
                # replicate to all 8 16-partition groups (tree doubling)
                for lo, n in ((16, 16), (32, 32), (64, 64)):
                    nc.sync.dma_start(out=w1[lo : lo + n, :, :], in_=w1[0:n, :, :])
                    nc.sync.dma_start(out=w2[lo : lo + n, :, :], in_=w2[0:n, :, :])

                # ---- chunked gather + combine + store ----
                w1v = w1.rearrange("p n r -> p (n r)")
                w2v = w2.rearrange("p n r -> p (n r)")
                for k in range(NCHUNK):
                    sl = slice(k * CCOL, (k + 1) * CCOL)
                    wsl = slice(k * CCOL * 8, (k + 1) * CCOL * 8)
                    gA = gpool.tile([NPART, CCOL, 64], f32, tag="gA", name="gA")
                    gB = gpool.tile([NPART, CCOL, 64], f32, tag="gB", name="gB")
                    nidx = NPART * CCOL
                    nc.gpsimd.dma_gather(
                        out_ap=gA[:], in_ap=tblv[:], idxs_ap=w1v[:, wsl],
                        num_idxs=nidx, num_idxs_reg=nidx, elem_size=64,
                        single_packet=False)
                    nc.gpsimd.dma_gather(
                        out_ap=gB[:], in_ap=tblv[:], idxs_ap=w2v[:, wsl],
                        num_idxs=nidx, num_idxs_reg=nidx, elem_size=64,
                        single_packet=False)

                    res = rpool.tile([NPART, CCOL, C], f32, tag="res", name="res")
                    tmp = rpool.tile([NPART, CCOL, C], f32, tag="tmp", name="tmp")
                    bshape = [NPART, CCOL, C]
                    first = True
                    for g, base_w in ((gA, 0), (gB, 3)):
                        for m in range(3):
                            wv = Wt[base_w + m][:, sl].to_broadcast(bshape)
                            if first:
                                nc.vector.tensor_mul(
                                    out=res[:], in0=g[:, :, 16 * m : 16 * m + 16],
                                    in1=wv)
                                first = False
                            else:
                                nc.vector.tensor_mul(
                                    out=tmp[:], in0=g[:, :, 16 * m : 16 * m + 16],
                                    in1=wv)
                                nc.vector.tensor_add(out=res[:], in0=res[:],
                                                     in1=tmp[:])
                    nc.sync.dma_start(out=out_r[b, :, sl, :], in_=res[:])
    nc.compile()
    return nc


def make_grids():
    # match jnp.linspace(-1, 1, n, dtype=f32): arange(n)*delta + start in f32
    def lin(n):
        delta = np.float32(2.0 / (n - 1))
        return (np.arange(n, dtype=np.float32) * delta + np.float32(-1.0)).astype(
            np.float32
        )

    xs = lin(OUT_W)
    ys = lin(OUT_H)
    # point t = p*NCOL + c  <-> grid position (p, c)
    t = np.arange(NPART, dtype=np.int64)[:, None] * NCOL + np.arange(NCOL)[None, :]
    ug = xs[t % OUT_W].astype(np.float32)
    vg = ys[t // OUT_W].astype(np.float32)
    return ug, vg


_PROGRAM = None


def _get_program():
    global _PROGRAM
    if _PROGRAM is None:
        _PROGRAM = build_program()
    return _PROGRAM


def _make_table(img: np.ndarray) -> np.ndarray:
    flat = np.ascontiguousarray(img).reshape(-1)
    t = np.empty(NENT * 64, dtype=np.float32)
    t[0 : HWPIX * C] = flat
    t[HWPIX * C : 2 * HWPIX * C - 32] = flat[32:]
    t[2 * HWPIX * C - 32 :] = 0.0
    return t.reshape(NENT, 64)


def make_in_maps(image: np.ndarray, transformation: np.ndarray):
    ug, vg = make_grids()
    in_maps = []
    for core in range(NCORES):
        in_maps.append(
            {
                **{
                    f"tbl{i}": _make_table(image[core * BLOC + i])
                    for i in range(BLOC)
                },
                "theta": np.tile(
                    np.ascontiguousarray(
                        transformation[core * BLOC : (core + 1) * BLOC]
                    ).reshape(1, BLOC * 6),
                    (NPART, 1),
                ),
                "ug": ug,
                "vg": vg,
            }
        )
    return in_maps


def run_spmd(image: np.ndarray, transformation: np.ndarray, **kwargs):
    nc = _get_program()
    in_maps = make_in_maps(image, transformation)
    res = run_bass_kernel_spmd(nc, in_maps, list(range(NCORES)), **kwargs)
    outs = [
        np.asarray(res.results[i]["out"]).reshape(BLOC, OUT_H, OUT_W, C)
        for i in range(NCORES)
    ]
    return np.concatenate(outs, axis=0), res


def kernel(image: np.ndarray, transformation: np.ndarray) -> np.ndarray:
    image = np.asarray(image, dtype=np.float32)
    transformation = np.asarray(transformation, dtype=np.float32)
    out, _ = run_spmd(image, transformation)
    return out
